# revision 13
# baseline (speedup 1.0000x reference)
"""Trainium2 Bass kernel for nn_MeshEstimator (SMPL-style mesh estimator).

Sharding: 8-way split of the vertex dimension R (6890 -> 7168 padded, 896/core).
The batch (B=256) is replicated; every core computes the full joint chain and
its own vertex shard. All input-dependent math runs on device; the host only
slices/stacks/pads inputs and reassembles outputs.

Self-contained: hardcodes all shapes; imports only numpy + concourse.
"""

import threading

import numpy as np

import concourse.bacc as bacc
import concourse.bass as bass
import concourse.tile as tile
import concourse.mybir as mybir
import concourse.bass_utils as bass_utils
from concourse.mybir import ActivationFunctionType as AF
from concourse.mybir import AluOpType as OP

F32 = mybir.dt.float32
PI = float(np.pi)

B = 256          # batch
T = 2            # batch tiles (256 = 2 * 128)
P = 128          # partitions
R = 6890         # vertices
NCORES = 8
RSH = 896        # vertex shard per core (7 tiles of 128)
RT = RSH // P    # 7 row-tiles per core
RPAD = RSH * NCORES  # 7168

PARENTS = np.array([0,0,0,0,1,2,3,4,5,6,7,8,9,9,9,12,13,14,16,17,18,19,20,21])
VERTS_LIST = np.array([1325,336,1032,4515,1374,4848,1739,5209,1960,5423])
SYNTH = np.array([3,15,4,5,7,8,18,19,20,21])

# kinematic-chain levels: (j0, n, parent0, parent_step) — all parent sets are
# affine in j which lets each level run as a handful of strided vector ops
LEVELS = [(1,3,0,0), (4,3,1,1), (7,3,4,1), (10,3,7,1), (13,2,9,0),
          (15,3,12,1), (18,2,16,1), (20,2,18,1), (22,2,20,1)]


def _bounds_np():
    Pp = np.pi
    b = [[-0.5933865286111969,0.5933865286111969],[-2*Pp,2*Pp],[-1.215762200416361,1.215762200416361],
         [-1.5793940868065197,0.3097956806],[-0.5881754611,0.5689768556],[-0.5323249722,0.6736965222],
         [-1.5793940868065197,0.3097956806],[-0.5689768556,0.5881754611],[-0.6736965222,0.5323249722],
         [-Pp/3,Pp/3],[-Pp/36,Pp/36],[-Pp/36,Pp/36],
         [-0.02268926111,2.441713561],[-0.01,0.01],[-0.01,0.01],
         [-0.02268926111,2.441713561],[-0.01,0.01],[-0.01,0.01],
         [-Pp/3,Pp/3],[-Pp/36,Pp/36],[-Pp/36,Pp/36]]
    b += [[-Pp/6,Pp/6]]*6
    b += [[-Pp/3,Pp/3],[-Pp/36,Pp/36],[-Pp/36,Pp/36]]
    b += [[-0.01,0.01]]*6
    b += [[-Pp/3,Pp/3],[-Pp/36,Pp/36],[-Pp/36,Pp/36]]
    k = [(-1.551596394,2.206094311),(-2.455676183,0.7627082389),(-1.570795,2.188641033),
         (-1.551596394,2.206094311),(-0.7627082389,2.455676183),(-2.188641033,1.570795)]
    b += [[a/3.0,c/3.0] for a,c in k]
    b += [[-Pp/3,Pp/3],[-Pp/36,Pp/36],[-Pp/36,Pp/36]]
    b += [[a*2.0/3.0,c*2.0/3.0] for a,c in k]
    b += [[-0.01,0.01],[-2.570867817,0.04799651389],[-0.01,0.01],[-0.01,0.01],[-0.04799651389,2.570867817],[-0.01,0.01]]
    b += [[-Pp/6,Pp/6]]*6
    b += [[-0.01,0.01]]*6
    return (2.0 * np.array(b, np.float32).astype(np.float32)).astype(np.float32)


def build_program(debug_taps=()):
    """Emit the Bass/Tile program. Returns (nc, debug_names)."""
    nc = bacc.Bacc("TRN2", target_bir_lowering=False, debug=False, num_devices=NCORES)

    # ---------------- DRAM I/O ----------------
    x_d = nc.dram_tensor("x", [B, 88], F32, kind="ExternalInput")
    g_d = nc.dram_tensor("g", [B, 2], F32, kind="ExternalInput")
    k1_d = nc.dram_tensor("k1", [3, 436, RSH], F32, kind="ExternalInput")
    w2a_d = nc.dram_tensor("w2a", [49, RSH], F32, kind="ExternalInput")
    jreg2_d = nc.dram_tensor("jreg2", [54, P, 48], F32, kind="ExternalInput")
    s2p_d = nc.dram_tensor("s2p", [54, P, 96], F32, kind="ExternalInput")
    cx_d = nc.dram_tensor("cx", [P, T, 88], F32, kind="ExternalInput")
    cmean_d = nc.dram_tensor("cmean", [P, T, 72], F32, kind="ExternalInput")
    cscale_d = nc.dram_tensor("cscale", [P, T, 72], F32, kind="ExternalInput")
    cscalei_d = nc.dram_tensor("cscalei", [P, T, 72], F32, kind="ExternalInput")
    cpf_d = nc.dram_tensor("cpf", [P, T, 216], F32, kind="ExternalInput")
    ident_d = nc.dram_tensor("ident", [P, P], F32, kind="ExternalInput")

    verts_d = nc.dram_tensor("verts", [RT, P, B * 3], F32, kind="ExternalOutput")
    betas_d = nc.dram_tensor("betas", [B, 10], F32, kind="ExternalOutput")
    pose_d = nc.dram_tensor("pose", [B, 72], F32, kind="ExternalOutput")
    rshift_d = nc.dram_tensor("rshift", [B, 3], F32, kind="ExternalOutput")
    rangles_d = nc.dram_tensor("rangles", [B, 6], F32, kind="ExternalOutput")
    newj_d = nc.dram_tensor("newj", [B, 72], F32, kind="ExternalOutput")

    taps = {}  # name -> (ap, shape) to dump as debug outputs

    def out_like(t, p):  # DRAM AP for a [256, c] per-batch tensor in [P, T, c] layout
        return t.ap().rearrange("(t p) c -> p t c", t=T)

    with tile.TileContext(nc) as tc:
        from contextlib import ExitStack
        with ExitStack() as ctx:
            const = ctx.enter_context(tc.tile_pool(name="const", bufs=1))
            main = ctx.enter_context(tc.tile_pool(name="main", bufs=1))
            tmpp = ctx.enter_context(tc.tile_pool(name="tmp", bufs=2))

            # ------------- const / input DMAs -------------
            ident = const.tile([P, P], F32)
            nc.sync.dma_start(out=ident, in_=ident_d.ap())
            cx = const.tile([P, T, 88], F32)
            nc.sync.dma_start(out=cx, in_=cx_d.ap())
            cmean = const.tile([P, T, 72], F32)
            nc.sync.dma_start(out=cmean, in_=cmean_d.ap())
            cscale = const.tile([P, T, 72], F32)
            nc.sync.dma_start(out=cscale, in_=cscale_d.ap())
            cscalei = const.tile([P, T, 72], F32)
            nc.sync.dma_start(out=cscalei, in_=cscalei_d.ap())
            cpf = const.tile([P, T, 24, 9], F32)
            nc.sync.dma_start(out=cpf, in_=cpf_d.ap().rearrange("p t (j c) -> p t j c", c=9))

            xs = main.tile([P, T, 88], F32)
            nc.sync.dma_start(out=xs, in_=x_d.ap().rearrange("(t p) c -> p t c", t=T))
            gs = main.tile([P, T, 2], F32)
            nc.sync.dma_start(out=gs, in_=g_d.ap().rearrange("(t p) c -> p t c", t=T))

            # big tables
            w2a = main.tile([49, RSH], F32)
            nc.sync.dma_start(out=w2a, in_=w2a_d.ap())
            k1t = []  # per-d list of chunk tiles
            KCH = [(0, 128), (128, 128), (256, 128), (384, 52)]
            for d in range(3):
                row = []
                for (c0, cn) in KCH:
                    t_ = main.tile([cn, RSH], F32, tag=f"k1_{d}_{c0}", name=f"k1_{d}_{c0}")
                    nc.sync.dma_start(out=t_, in_=k1_d.ap()[d, c0:c0 + cn, :])
                    row.append(t_)
                k1t.append(row)
            jreg2 = main.tile([P, 54, 48], F32)
            nc.sync.dma_start(out=jreg2, in_=jreg2_d.ap().rearrange("n p c -> p n c"))
            s2p = main.tile([P, 54, 96], F32)
            nc.sync.dma_start(out=s2p, in_=s2p_d.ap().rearrange("n p c -> p n c"))

            # ------------- preprocessing (batch on partitions) -------------
            xadj = main.tile([P, T, 88], F32)
            nc.vector.tensor_add(xadj, xs[:], cx[:])

            betas = main.tile([P, T, 10], F32)
            nc.scalar.activation(out=betas, in_=xadj[:, :, 0:10], func=AF.Tanh, scale=1.0 / 3.0)
            nc.vector.tensor_scalar_mul(betas[:], betas[:], 3.0)

            # atan2(y, x): y = x[:,16:19], x = x[:,13:16]
            att = main.tile([P, T, 3], F32, tag="att")
            ax = xadj[:, :, 13:16]
            ay = xadj[:, :, 16:19]
            rx = tmpp.tile([P, T, 3], F32, tag="rx")
            nc.vector.reciprocal(out=rx, in_=ax)
            q_ = tmpp.tile([P, T, 3], F32, tag="q_")
            nc.vector.tensor_mul(q_, ay, rx[:])
            nc.scalar.activation(out=att, in_=q_[:], func=AF.Arctan)
            xneg = tmpp.tile([P, T, 3], F32, tag="xneg")
            nc.vector.tensor_scalar(xneg, ax, 0.0, None, OP.is_lt)
            m2 = tmpp.tile([P, T, 3], F32, tag="m2")
            nc.vector.tensor_scalar(m2, ay, 0.0, None, OP.is_ge)
            nc.vector.tensor_scalar(m2[:], m2[:], 2.0, -1.0, OP.mult, OP.add)
            corr = tmpp.tile([P, T, 3], F32, tag="corr")
            nc.vector.tensor_mul(corr, xneg[:], m2[:])
            rot = main.tile([P, T, 3], F32, tag="rot")
            nc.vector.scalar_tensor_tensor(rot, corr[:], PI, att[:], OP.mult, OP.add)

            # pose (clamped) [P, T, 72]
            pose = main.tile([P, T, 72], F32)
            nc.scalar.copy(pose[:, :, 0:3], rot[:])
            nc.scalar.copy(pose[:, :, 3:72], xadj[:, :, 19:88])
            nc.vector.tensor_tensor(out=pose[:], in0=pose[:], in1=cmean[:], op=OP.subtract)
            nc.vector.tensor_mul(pose[:], pose[:], cscale[:])
            nc.scalar.activation(out=pose[:], in_=pose[:], func=AF.Tanh)
            nc.vector.tensor_mul(pose[:], pose[:], cscalei[:])
            nc.vector.tensor_add(pose[:], pose[:], cmean[:])

            # ------------- Rodrigues -------------
            pj = pose[:].rearrange("p t (j d) -> p t j d", d=3)
            tp = tmpp.tile([P, T, 24, 3], F32, tag="tp")
            nc.vector.tensor_scalar_add(tp, pj, 1e-8)
            sq = tmpp.tile([P, T, 24, 3], F32, tag="sq")
            nc.scalar.activation(out=sq, in_=tp[:], func=AF.Square)
            a2 = tmpp.tile([P, T, 24], F32, tag="a2")
            nc.vector.tensor_add(a2, sq[:, :, :, 0], sq[:, :, :, 1])
            nc.vector.tensor_add(a2[:], a2[:], sq[:, :, :, 2])
            ang = main.tile([P, T, 24], F32, tag="ang")
            nc.scalar.activation(out=ang, in_=a2[:], func=AF.Sqrt)
            half = tmpp.tile([P, T, 24], F32, tag="half")
            nc.vector.tensor_scalar_mul(half, ang[:], 0.5)
            # range-reduce half into [-pi/2, pi/2]: two conditional pi-subtractions
            s1 = tmpp.tile([P, T, 24], F32, tag="s1")
            nc.vector.tensor_scalar(s1, half[:], PI / 2, None, OP.is_gt)
            h1 = tmpp.tile([P, T, 24], F32, tag="h1")
            nc.vector.scalar_tensor_tensor(h1, s1[:], -PI, half[:], OP.mult, OP.add)
            s2_ = tmpp.tile([P, T, 24], F32, tag="s2_")
            nc.vector.tensor_scalar(s2_, h1[:], PI / 2, None, OP.is_gt)
            hr = tmpp.tile([P, T, 24], F32, tag="hr")
            nc.vector.scalar_tensor_tensor(hr, s2_[:], -PI, h1[:], OP.mult, OP.add)
            # parity -> sign = 1 - 2*(s1 xor s2)
            px = tmpp.tile([P, T, 24], F32, tag="px")
            nc.vector.tensor_mul(px, s1[:], s2_[:])
            sadd = tmpp.tile([P, T, 24], F32, tag="sadd")
            nc.vector.tensor_add(sadd, s1[:], s2_[:])
            nc.vector.scalar_tensor_tensor(sadd[:], px[:], -2.0, sadd[:], OP.mult, OP.add)
            sgn = tmpp.tile([P, T, 24], F32, tag="sgn")
            nc.vector.tensor_scalar(sgn, sadd[:], -2.0, 1.0, OP.mult, OP.add)
            sh = tmpp.tile([P, T, 24], F32, tag="sh")
            nc.scalar.activation(out=sh, in_=hr[:], func=AF.Sin)
            nc.vector.tensor_mul(sh[:], sh[:], sgn[:])
            halfpi = const.tile([P, 1], F32)
            nc.gpsimd.memset(halfpi[:], PI / 2)
            ch = tmpp.tile([P, T, 24], F32, tag="ch")
            nc.scalar.activation(out=ch, in_=hr[:], func=AF.Sin, bias=halfpi[:])
            nc.vector.tensor_mul(ch[:], ch[:], sgn[:])

            rinv = tmpp.tile([P, T, 24], F32, tag="rinv")
            nc.vector.reciprocal(out=rinv, in_=ang[:])
            soa = tmpp.tile([P, T, 24], F32, tag="soa")
            nc.vector.tensor_mul(soa, sh[:], rinv[:])
            qv = main.tile([P, T, 24, 3], F32, tag="qv")
            nc.vector.tensor_mul(qv, pj, soa[:].unsqueeze(3).broadcast_to([P, T, 24, 3]))
            # normalize quaternion (w=ch, v=qv)
            cw2 = tmpp.tile([P, T, 24], F32, tag="cw2")
            nc.scalar.activation(out=cw2, in_=ch[:], func=AF.Square)
            sq2 = tmpp.tile([P, T, 24, 3], F32, tag="sq2")
            nc.scalar.activation(out=sq2, in_=qv[:], func=AF.Square)
            qn2 = tmpp.tile([P, T, 24], F32, tag="qn2")
            nc.vector.tensor_add(qn2, sq2[:, :, :, 0], sq2[:, :, :, 1])
            nc.vector.tensor_add(qn2[:], qn2[:], sq2[:, :, :, 2])
            nc.vector.tensor_add(qn2[:], qn2[:], cw2[:])
            qn = tmpp.tile([P, T, 24], F32, tag="qn")
            nc.scalar.activation(out=qn, in_=qn2[:], func=AF.Sqrt)
            qni = tmpp.tile([P, T, 24], F32, tag="qni")
            nc.vector.reciprocal(out=qni, in_=qn[:])
            w_ = main.tile([P, T, 24], F32, tag="w_")
            nc.vector.tensor_mul(w_, ch[:], qni[:])
            nc.vector.tensor_mul(qv[:], qv[:], qni[:].unsqueeze(3).broadcast_to([P, T, 24, 3]))

            # quaternion products
            def prod(name, a_ap, b_ap):
                t_ = tmpp.tile([P, T, 24], F32, tag=name)
                nc.vector.tensor_mul(t_, a_ap, b_ap)
                return t_
            vx, vy, vz = qv[:, :, :, 0], qv[:, :, :, 1], qv[:, :, :, 2]
            xx = prod("xx", vx, vx); yy = prod("yy", vy, vy); zz = prod("zz", vz, vz)
            xy = prod("xy", vx, vy); xz = prod("xz", vx, vz); yz = prod("yz", vy, vz)
            wx = prod("wx", w_[:], vx); wy = prod("wy", w_[:], vy); wz = prod("wz", w_[:], vz)

            Rs = main.tile([P, T, 24, 3, 3], F32)
            tt24 = tmpp.tile([P, T, 24], F32, tag="tt24")
            def diag(rc, p1, p2):
                nc.vector.tensor_add(tt24[:], p1[:], p2[:])
                nc.vector.tensor_scalar(Rs[:, :, :, rc, rc], tt24[:], -2.0, 1.0, OP.mult, OP.add)
            def offd(r_, c_, p1, p2, op2):
                nc.vector.tensor_tensor(out=tt24[:], in0=p1[:], in1=p2[:], op=op2)
                nc.vector.tensor_scalar_mul(Rs[:, :, :, r_, c_], tt24[:], 2.0)
            diag(0, yy, zz); diag(1, xx, zz); diag(2, xx, yy)
            offd(0, 1, xy, wz, OP.subtract); offd(0, 2, xz, wy, OP.add)
            offd(1, 0, xy, wz, OP.add);      offd(1, 2, yz, wx, OP.subtract)
            offd(2, 0, xz, wy, OP.subtract); offd(2, 1, yz, wx, OP.add)

            # ------------- coefP / coefS -------------
            rsm = main.tile([P, T, 24, 9], F32)  # Rs - I (joints >= 1)
            nc.vector.tensor_add(rsm, Rs[:].rearrange("p t j a b -> p t j (a b)"), cpf[:])

            coefS = main.tile([P, T, 22], F32)
            nc.scalar.copy(coefS[:, :, 0:2], gs[:])
            for g in range(2):
                nc.vector.tensor_mul(coefS[:, :, 2 + 10 * g: 12 + 10 * g], betas[:],
                                     gs[:, :, g:g + 1].broadcast_to([P, T, 10]))
            coefP = main.tile([P, T, 436], F32)
            nc.scalar.copy(coefP[:, :, 0:22], coefS[:])
            pf = rsm[:, :, 1:24, :].rearrange("p t j c -> p t (j c)")  # 207
            for g in range(2):
                nc.vector.tensor_mul(coefP[:, :, 22 + 207 * g: 229 + 207 * g], pf,
                                     gs[:, :, g:g + 1].broadcast_to([P, T, 207]))

            # ------------- transposes: coefS_T, coefP_T -------------
            # coefS replicated 3x along free dim so its transpose lands at
            # partition bases 0/32/64 (matmul operands must share partitions)
            coefS3 = main.tile([P, T, 3, 32], F32)
            nc.gpsimd.memset(coefS3[:], 0.0)
            for d3 in range(3):
                nc.scalar.copy(coefS3[:, :, d3, 0:22], coefS[:])
            coefST = main.tile([96, B], F32)
            cpT = [main.tile([cn, B], F32, tag=f"cpT{c0}", name=f"cpT{c0}") for (c0, cn) in KCH]

            with tc.tile_pool(name="ps_tr", bufs=2, space="PSUM") as ps_tr:
                for t in range(T):
                    pt = ps_tr.tile([96, P], F32, tag="tr")
                    nc.tensor.transpose(
                        pt, coefS3[:, t].rearrange("p a b -> p (a b)"), ident[:])
                    nc.scalar.copy(coefST[:, t * P:(t + 1) * P], pt[:])
                for t in range(T):
                    for ci, (c0, cn) in enumerate(KCH):
                        pt = ps_tr.tile([cn, P], F32, tag="tr")
                        nc.tensor.transpose(pt, coefP[:, t, c0:c0 + cn], ident[:])
                        nc.scalar.copy(cpT[ci][:, t * P:(t + 1) * P], pt[:])

                # ------------- J precompute: jp[96, 48] = sum_r s2p^T jreg2 -------------
                jp_sb = main.tile([96, 48], F32)
                with tc.tile_pool(name="ps_jp", bufs=1, space="PSUM") as ps_jp:
                    jp_ps = ps_jp.tile([96, 48], F32)
                    for i in range(54):
                        nc.tensor.matmul(jp_ps, s2p[:, i, :], jreg2[:, i, :],
                                         start=(i == 0), stop=(i == 53))
                    nc.scalar.copy(jp_sb[:], jp_ps[:])

                # ------------- J3_d[(g,j), b] and J in batch layout -------------
                J = main.tile([P, T, 24, 3], F32)
                with tc.tile_pool(name="ps_j3", bufs=3, space="PSUM") as ps_j3:
                    for d in range(3):
                        j3 = ps_j3.tile([48, B], F32, tag="j3t")
                        nc.tensor.matmul(j3, jp_sb[32 * d:32 * d + 32, :],
                                         coefST[32 * d:32 * d + 32, :],
                                         start=True, stop=True)
                        j3s = tmpp.tile([48, B], F32, tag="j3s")
                        nc.scalar.copy(j3s[:], j3[:])
                        for t in range(T):
                            jt = ps_j3.tile([P, 48], F32, tag="j3t")
                            nc.tensor.transpose(jt, j3s[:, t * P:(t + 1) * P], ident[0:48, 0:48])
                            nc.vector.tensor_scalar(J[:, t, :, d], jt[:, 0:24],
                                                    gs[:, t, 0:1], None, OP.mult)
                            nc.vector.scalar_tensor_tensor(J[:, t, :, d], jt[:, 24:48],
                                                           gs[:, t, 1:2], J[:, t, :, d],
                                                           OP.mult, OP.add)

                # ------------- A matrices + kinematic chain (batch layout) -------------
                Jpar = main.tile([P, T, 24, 3], F32)
                nc.gpsimd.memset(Jpar[:, :, 0:1, :], 0.0)
                for (j0, n, p0, ps_) in LEVELS:
                    if ps_ == 0:
                        src = J[:, :, p0:p0 + 1, :].broadcast_to([P, T, n, 3])
                    else:
                        src = J[:, :, p0:p0 + n, :]
                    nc.scalar.copy(Jpar[:, :, j0:j0 + n, :], src)
                tcol = main.tile([P, T, 24, 3], F32)
                nc.vector.tensor_tensor(out=tcol, in0=J[:], in1=Jpar[:], op=OP.subtract)

                A = main.tile([P, T, 24, 4, 4], F32)
                nc.gpsimd.memset(A[:], 0.0)
                for t in range(T):
                    nc.scalar.copy(A[:, t, :, 0:3, 0:3], Rs[:, t])
                nc.scalar.copy(A[:, :, :, 0:3, 3:4], tcol[:].unsqueeze(4))
                nc.vector.memset(A[:, :, :, 3:4, 3:4], 1.0)

                G = main.tile([P, T, 24, 4, 4], F32)
                for t in range(T):
                    nc.scalar.copy(G[:, t, 0:1, :, :], A[:, t, 0:1, :, :])
                gtmp = main.tile([P, T, 3, 4, 4], F32)
                for (j0, n, p0, ps_) in LEVELS:
                    for t in range(T):
                        for m in range(4):
                            if ps_ == 0:
                                gp = G[:, t, p0:p0 + 1, :, m:m + 1].broadcast_to([P, n, 4, 4])
                            else:
                                gp = G[:, t, p0:p0 + n, :, m:m + 1].broadcast_to([P, n, 4, 4])
                            am = A[:, t, j0:j0 + n, m:m + 1, :].broadcast_to([P, n, 4, 4])
                            if m == 0:
                                nc.vector.tensor_mul(G[:, t, j0:j0 + n, :, :], gp, am)
                            else:
                                nc.vector.tensor_mul(gtmp[:, t, 0:n, :, :], gp, am)
                                nc.vector.tensor_add(G[:, t, j0:j0 + n, :, :],
                                                     G[:, t, j0:j0 + n, :, :],
                                                     gtmp[:, t, 0:n, :, :])

                # new_J = G_trans + (root_shift - J0)
                off = main.tile([P, T, 3], F32)
                nc.vector.tensor_tensor(out=off, in0=xadj[:, :, 10:13],
                                        in1=J[:, :, 0, :], op=OP.subtract)
                newj = main.tile([P, T, 24, 3], F32)
                nc.vector.tensor_add(newj, G[:, :, :, 0:3, 3],
                                     off[:].unsqueeze(2).broadcast_to([P, T, 24, 3]))

                # A_bar: G_trans -= G_rot @ J_j   (in place on G)
                grj = main.tile([P, T, 24, 3], F32)
                gr2 = main.tile([P, T, 24, 3], F32)
                nc.vector.tensor_mul(grj, G[:, :, :, 0:3, 0],
                                     J[:, :, :, 0:1].broadcast_to([P, T, 24, 3]))
                nc.vector.tensor_mul(gr2, G[:, :, :, 0:3, 1],
                                     J[:, :, :, 1:2].broadcast_to([P, T, 24, 3]))
                nc.vector.tensor_add(grj[:], grj[:], gr2[:])
                nc.vector.tensor_mul(gr2[:], G[:, :, :, 0:3, 2],
                                     J[:, :, :, 2:3].broadcast_to([P, T, 24, 3]))
                nc.vector.tensor_add(grj[:], grj[:], gr2[:])
                nc.vector.tensor_tensor(out=G[:, :, :, 0:3, 3], in0=G[:, :, :, 0:3, 3],
                                        in1=grj[:], op=OP.subtract)

                # ------------- Ag2_pre: gender-scaled transforms + aug row -------------
                ag2p = main.tile([P, T, 49, 3, 4], F32)
                for t in range(T):
                    for g in range(2):
                        nc.vector.tensor_scalar(ag2p[:, t, 24 * g:24 * g + 24, :, :],
                                                G[:, t, :, 0:3, :], gs[:, t, g:g + 1],
                                                None, OP.mult)
                nc.gpsimd.memset(ag2p[:, :, 48, :, :], 0.0)
                nc.scalar.copy(ag2p[:, :, 48, :, 3:4], off[:].unsqueeze(3))

                # transpose to Ag2_sb[49, b, pq]
                ag2 = main.tile([49, B, 12], F32)
                for t in range(T):
                    for pq in range(12):
                        ri, cd = pq // 4, pq % 4
                        pt = ps_tr.tile([49, P], F32, tag="tr")
                        nc.tensor.transpose(pt, ag2p[:, t, :, ri, cd], ident[:])
                        nc.scalar.copy(ag2[:, t * P:(t + 1) * P, pq:pq + 1], pt[:].unsqueeze(2))

                # ------------- stage V: vp = coefP @ K1 (per rtile, per d) -------------
                vp = main.tile([P, RT, T, 4, 32, 3], F32)  # [p, rt, (b), d]
                with tc.tile_pool(name="ps_s1", bufs=3, space="PSUM") as ps_s1:
                    for rt in range(RT):
                        for d in range(3):
                            vps = ps_s1.tile([P, B], F32, tag="vps")
                            for ci, (c0, cn) in enumerate(KCH):
                                nc.tensor.matmul(vps, k1t[d][ci][:, rt * P:(rt + 1) * P],
                                                 cpT[ci][:], start=(ci == 0), stop=(ci == 3))
                            nc.scalar.copy(vp[:, rt, :, :, :, d:d + 1],
                                           vps[:].rearrange("p (t c b) -> p t c b",
                                                            t=T, c=4).unsqueeze(4))

            # ------------- LBS: T = W2a^T @ Ag2 ; verts = T[:, :3,:3] @ vp + T[:,:3,3] -------------
            verts_pool = ctx.enter_context(tc.tile_pool(name="verts", bufs=3))
            fma_pool = ctx.enter_context(tc.tile_pool(name="fma", bufs=3))
            with tc.tile_pool(name="ps_T", bufs=2, space="PSUM") as ps_T:
                for rt in range(RT):
                    vt_ = verts_pool.tile([P, T, 4, 32, 3], F32, tag="vt")
                    for grp in range(T):
                        tps = ps_T.tile([P, 4, 32, 16], F32, tag="tps")
                        for bc in range(4):
                            b0 = grp * 128 + bc * 32
                            nc.tensor.matmul(tps[:, bc, :, 0:12],
                                             w2a[:, rt * P:(rt + 1) * P],
                                             ag2[:, b0:b0 + 32, :], start=True, stop=True)
                        ta = fma_pool.tile([P, 4, 32, 3], F32, tag="ta")
                        tb = fma_pool.tile([P, 4, 32, 3], F32, tag="tb")
                        def T_(d):  # T[:, bc, b, (i,d)] at fixed d -> dims (bc, b, i)
                            return tps[:, :, :, d:12:4]
                        def V_(d):
                            return vp[:, rt, grp, :, :, d:d + 1].broadcast_to([P, 4, 32, 3])
                        nc.vector.tensor_mul(ta, T_(0), V_(0))
                        nc.vector.tensor_mul(tb, T_(1), V_(1))
                        nc.vector.tensor_add(ta[:], ta[:], tb[:])
                        nc.vector.tensor_mul(tb[:], T_(2), V_(2))
                        nc.vector.tensor_add(ta[:], ta[:], tb[:])
                        nc.vector.tensor_add(vt_[:, grp, :, :, :], ta[:], T_(3))
                    nc.sync.dma_start(
                        out=verts_d.ap()[rt].rearrange("p (t c b i) -> p t c b i",
                                                       t=T, c=4, b=32),
                        in_=vt_[:])

            # ------------- small outputs -------------
            nc.sync.dma_start(out=out_like(betas_d, P), in_=betas[:])
            nc.sync.dma_start(out=out_like(pose_d, P), in_=pose[:])
            nc.sync.dma_start(out=out_like(rshift_d, P), in_=xadj[:, :, 10:13])
            nc.sync.dma_start(out=out_like(rangles_d, P), in_=xadj[:, :, 13:19])
            nc.sync.dma_start(out=out_like(newj_d, P),
                              in_=newj[:].rearrange("p t j d -> p t (j d)"))

            # ------------- debug taps -------------
            tapsrc = {
                "xadj": (xadj, [P, T, 88]), "rot": (rot, [P, T, 3]),
                "pose_t": (pose, [P, T, 72]), "Rs": (Rs, [P, T, 24, 3, 3]),
                "coefP": (coefP, [P, T, 436]), "J": (J, [P, T, 24, 3]),
                "G": (G, [P, T, 24, 4, 4]), "ag2": (ag2, [49, B, 12]),
                "jp": (jp_sb, [96, 48]), "coefST": (coefST, [32, B]),
            }
            for name in debug_taps:
                tl, shp = tapsrc[name]
                flat = int(np.prod(shp[1:]))
                dt_ = nc.dram_tensor(f"dbg_{name}", [shp[0], flat], F32, kind="ExternalOutput")
                src = tl[:]
                if len(shp) > 2:
                    letters = [f"a{i}" for i in range(len(shp) - 1)]
                    src = src.rearrange(f"p {' '.join(letters)} -> p ({' '.join(letters)})")
                nc.sync.dma_start(out=dt_.ap(), in_=src)

    nc.compile()
    return nc


def host_prep(inputs):
    """Slice/stack/pad inputs into per-core in_maps (no input-dependent math)."""
    x = np.ascontiguousarray(inputs["x"], np.float32)
    g = np.ascontiguousarray(inputs["batch_gender"], np.float32)
    vt = np.asarray(inputs["v_template"], np.float32)    # [2, R, 3]
    sd = np.asarray(inputs["shapedirs"], np.float32)     # [2, 10, R, 3]
    jr = np.asarray(inputs["J_regressor"], np.float32)   # [2, R, 24]
    pd = np.asarray(inputs["posedirs"], np.float32)      # [2, R, 3, 207]
    wt = np.asarray(inputs["weights"], np.float32)       # [2, R, 24]

    # shared tables
    NR = 54 * P  # 6912
    jreg2 = np.zeros((NR, 48), np.float32)
    jreg2[:R, 0:24] = jr[0]
    jreg2[:R, 24:48] = jr[1]
    jreg2 = jreg2.reshape(54, P, 48)
    s2p = np.zeros((NR, 96), np.float32)
    for d in range(3):
        c = 32 * d
        s2p[:R, c + 0] = vt[0, :, d]
        s2p[:R, c + 1] = vt[1, :, d]
        for gg in range(2):
            s2p[:R, c + 2 + 10 * gg: c + 12 + 10 * gg] = sd[gg, :, :, d].T
    s2p = s2p.reshape(54, P, 96)

    bounds = _bounds_np()
    mean_b = bounds.mean(axis=1).astype(np.float32)
    scale = (2.0 / np.abs(bounds[:, 0] - bounds[:, 1])).astype(np.float32)
    scalei = (1.0 / scale).astype(np.float32)

    cxv = np.zeros((88,), np.float32)
    cxv[10] = np.float32(0.6 - 0.286)
    cxv[11] = np.float32(1.2 - 0.286)
    cxv[12] = np.float32(0.1)
    cx = np.broadcast_to(cxv, (P, T, 88)).copy()
    cmean = np.broadcast_to(mean_b, (P, T, 72)).copy()
    cscale = np.broadcast_to(scale, (P, T, 72)).copy()
    cscalei = np.broadcast_to(scalei, (P, T, 72)).copy()
    cpfv = np.zeros((24, 9), np.float32)
    cpfv[1:, 0] = -1.0; cpfv[1:, 4] = -1.0; cpfv[1:, 8] = -1.0
    cpf = np.broadcast_to(cpfv.reshape(216), (P, T, 216)).copy()
    ident = np.eye(P, dtype=np.float32)

    shared = dict(x=x, g=g, jreg2=jreg2, s2p=s2p, cx=cx, cmean=cmean,
                  cscale=cscale, cscalei=cscalei, cpf=cpf, ident=ident)

    in_maps = []
    for c in range(NCORES):
        r0 = c * RSH
        r1 = min(r0 + RSH, R)
        n = r1 - r0
        k1 = np.zeros((3, 436, RSH), np.float32)
        w2a = np.zeros((49, RSH), np.float32)
        if n > 0:
            for d in range(3):
                k1[d, 0, :n] = vt[0, r0:r1, d]
                k1[d, 1, :n] = vt[1, r0:r1, d]
                for gg in range(2):
                    k1[d, 2 + 10 * gg: 12 + 10 * gg, :n] = sd[gg, :, r0:r1, d]
                    k1[d, 22 + 207 * gg: 229 + 207 * gg, :n] = pd[gg, r0:r1, d, :].T
            w2a[0:24, :n] = wt[0, r0:r1, :].T
            w2a[24:48, :n] = wt[1, r0:r1, :].T
        w2a[48, :] = 1.0
        in_maps.append({**shared, "k1": k1, "w2a": w2a})
    return in_maps


_CACHE = {}
_LOCK = threading.Lock()


def _get_nc(debug_taps=()):
    key = tuple(debug_taps)
    with _LOCK:
        if key not in _CACHE:
            _CACHE[key] = build_program(debug_taps=key)
        return _CACHE[key]


def run_device(inputs, debug_taps=()):
    nc = _get_nc(debug_taps)
    in_maps = host_prep(inputs)
    res = bass_utils.run_bass_kernel_spmd(nc, in_maps, core_ids=list(range(NCORES)))
    return res.results


def assemble(results):
    """Host gather: shards -> full outputs, plus the two sliced outputs."""
    r0 = results[0]
    betas = r0["betas"]
    pose = r0["pose"]
    root_shift = r0["rshift"]
    root_angles = r0["rangles"]
    new_J = r0["newj"].reshape(B, 24, 3)
    shards = []
    for c in range(NCORES):
        v = results[c]["verts"].reshape(RT, P, B, 3)     # [rt, p, b, i]
        shards.append(v.transpose(2, 0, 1, 3).reshape(B, RSH, 3))
    verts = np.concatenate(shards, axis=1)[:, :R, :]
    verts_red = verts[:, VERTS_LIST, :]
    verts_offset = verts_red - new_J[:, SYNTH, :]
    return (betas, pose, root_shift, root_angles, verts, verts_red, new_J,
            verts_offset)


def kernel(**inputs):
    return assemble(run_device(inputs))
